# revision 19
# baseline (speedup 1.0000x reference)
"""Trainium2 Bass kernel for nn_DGLossVer2 (gyro Huber loss + gaussian NLL).

Strategy (v2)
-------------
Data-parallel over batch N=128 across 8 NeuronCores (16 sequences/core,
8 partitions per sequence, L=2048 steps per partition).

Host prep (layout/dtype only, no arithmetic): every stream is cast to
fp16 and laid out planar per chunk: [P, nch, 3, Cs].  dw_16 is
subsampled host-side (dw_16[:, ::16], pure indexing) and its 16-groups
are stored even|odd-split so the level-5 pairwise quat product reads
contiguous halves (keeps DVE 2x_1p mode).  fp16 end-to-end was
validated against the f32 reference in numpy at rel err 2.8e-5.

Engine split per chunk (Cs=512 steps, 1536 elems):
  DVE : Sc=max(sd,1e-3); d1=gt-wh; d=d1-mn; u=d*isd (all fp16 2x);
        level-3/4 tree reduce (TR of 4); TTR partial sum of u^2
  ACT : lnS=Ln(Sc) [+accum]; isd=Exp(-lnS); Square-accum of u^2 part
  Pool: pairwise halve levels 1+2 of the w_hat 16-sum tree
Tail: hat quats by 5th-order Taylor (f32 math, fp16 storage); gt quats
from dw (Sqrt/Sin); q32 = q16e*q16o and residual conj(hat)*gt in fp16
(2x); theta = 2*atan2(|v|, w) with w=cos(theta/2)>0 always, via min/max
reflection; Huber m*(2|t|-m) partial-summed with tensor_tensor_reduce.
Host combines per-partition partials in f64.
"""

import numpy as np

import concourse.bass as bass
import concourse.mybir as mybir
from concourse.mybir import AluOpType as Op
from concourse.mybir import ActivationFunctionType as AF
from concourse.tile import TileContext

F32 = mybir.dt.float32
F16 = mybir.dt.float16
AX = mybir.AxisListType


def _patch_drain():
    """walrus codegen in this container rejects >1 sync wait on SP-engine
    instructions; spread the kernel-tail drain's waits across 1-wait NOPs."""
    from concourse import tile as tile_mod
    from concourse.vector_clock import ScopedClock

    if getattr(tile_mod.TileContext, "_drain_patched", False):
        return

    def _drain_and_barrier(self, tick_clock, wait_clock):
        nop0 = self.nc.sync.nop(nofuse=True)
        wait_clock.add_sem_waits(nop0.ins,
                                 ScopedClock({None: tick_clock.global_clock}))
        si = nop0.ins.sync_info
        if si is not None and len(si.on_wait) > 1:
            waits = list(si.on_wait)
            si.on_wait = waits[:1]
            for w in waits[1:]:
                nopn = self.nc.sync.nop(nofuse=True)
                nopn.ins.sync_info = mybir.SyncInfo(on_wait=[w], on_update=[])
        self.nc.sync.drain()
        self.nc.all_engine_barrier()
        assert self.sems is not None
        popped = self.nc._tile_sem_poison_stack.pop()
        assert popped is self._sem_poison
        self.nc.clear_and_free_semaphores(list(self.sems.allocated().values()))
        self.nc.all_engine_barrier()

    tile_mod.TileContext._drain_and_barrier = _drain_and_barrier
    tile_mod.TileContext._drain_patched = True


def _split_multi_waits(nc):
    """This container's walrus codegen allows only one sync wait per
    instruction; move extra waits onto same-engine NoOps inserted before."""
    n = 0
    for bb in nc.m.functions[0].blocks:
        new = []
        for inst in bb.instructions:
            si = inst.sync_info
            if si is not None and len(si.on_wait) > 1:
                waits = list(si.on_wait)
                for w in waits[:-1]:
                    n += 1
                    new.append(mybir.InstNoOp(
                        name=f"wsplit-{n}", engine=inst.engine,
                        sync_info=mybir.SyncInfo(on_wait=[w], on_update=[]),
                        bass_nofuse=True))
                si.on_wait = waits[-1:]
            new.append(inst)
        bb.instructions[:] = new
    return n


DT = 0.005
W_ = 1.0e6
H_ = 0.005
N0 = 5
PI = float(np.pi)

N_CORES = 8
N_FULL, T_FULL = 128, 16384
P = 128
NSEQ = 16               # sequences per core
SP = P // NSEQ          # partitions per sequence (8)
L = T_FULL // SP        # steps per partition (2048)
NCH = 4                 # chunks
CS = L // NCH           # steps per partition per chunk (512)
N16 = L // 16           # 128 16-groups per partition
N32 = L // 32           # 64
NCAT = N16 + N32        # 192


def build():
    _patch_drain()
    nc = bass.Bass()
    for cname, cval in (("pi2", PI / 2), ("pi", PI), ("tiny", 1e-30),
                        ("m1", -1.0)):
        _cc = nc.alloc_sbuf_tensor(f"const-f32-{cname}", [128, 1], F32)
        nc.gpsimd.memset(_cc.ap(), cval)
        nc.const_aps.aps[(F32, cval)] = _cc.ap()
    nc.all_engine_barrier()

    CW = 3 * CS          # elems per chunk (1536)
    wh_d = nc.declare_dram_parameter("w_hat", [P, NCH * CW], F16, isOutput=False)
    gt_d = nc.declare_dram_parameter("w_gt", [P, NCH * CW], F16, isOutput=False)
    mn_d = nc.declare_dram_parameter("w_mean", [P, NCH * CW], F16, isOutput=False)
    sd_d = nc.declare_dram_parameter("w_std", [P, NCH * CW], F16, isOutput=False)
    dw_d = nc.declare_dram_parameter("dw_16", [P, 3 * N16], F16, isOutput=False)
    mkc_d = nc.declare_dram_parameter("maskc", [P, NCAT], F32, isOutput=False)
    out_d = nc.declare_dram_parameter("out", [P, 4], F32, isOutput=True)

    from contextlib import ExitStack
    with TileContext(nc) as tc, ExitStack() as _es:
        v = nc.vector
        g = nc.gpsimd
        act = nc.scalar
        pp = _es.enter_context(tc.tile_pool(name="persist", bufs=1))

        def ptile(shape, name, dtype=F32):
            return pp.tile(shape, dtype, name=name, tag=name)

        # persistent planes
        # scat: interleaved [g, c] 16|32-group sums (f32)
        scat = ptile([P, 3 * NCAT], "scat")
        scat_v = scat[:].rearrange("p (g c) -> p g c", c=3)
        dw_t = ptile([P, 3 * N16], "dw_t", F16)
        gq = [ptile([P, NCAT], f"gq{i}", F16) for i in range(4)]  # gt quats
        hq = [ptile([P, NCAT], f"hq{i}", F16) for i in range(4)]  # hat quats
        rqw = ptile([P, NCAT], "rqw", F16)          # residual w
        rqv = ptile([P, 3 * NCAT], "rqv", F16)      # residual xyz comp-major
        s16 = [ptile([P, NCAT], f"s16_{i}", F16) for i in range(3)]  # qmul scr
        mkc_t = ptile([P, NCAT], "mkc")
        acc_ln = ptile([P, NCH], "acc_ln")
        acc_u2a = ptile([P, NCH], "acc_u2a")
        acc16 = ptile([P, 3], "acc16")
        acc32 = ptile([P, 3], "acc32")
        # f32 scratch planes; pxa..pxc are Pool-private
        fa = ptile([P, 3 * NCAT], "fa")
        fb = ptile([P, 3 * NCAT], "fb")
        fc = ptile([P, 2 * NCAT], "fc")
        fd = ptile([P, 2 * NCAT], "fd")
        pxa = ptile([P, NCAT], "pxa")
        pxb = ptile([P, NCAT], "pxb")
        pxc = ptile([P, NCAT], "pxc")

        nc.sync.dma_start(out=mkc_t[:], in_=mkc_d[:])
        nc.sync.dma_start(out=dw_t[:], in_=dw_d[:])

        def dma4(tile_ap, dram_ap, k=4):
            step = P // k
            for i_ in range(k):
                psl = slice(i_ * step, (i_ + 1) * step)
                nc.sync.dma_start(out=tile_ap[psl, :], in_=dram_ap[psl, :])

        # ------------- dw -> gt quats (before chunk loop; sqrt+trig) -------
        dsq = fa[:, :3 * N16]
        v.tensor_tensor(dsq, dw_t[:], dw_t[:], Op.mult)
        da2 = fb[:, :N16]
        v.tensor_reduce(da2, dsq.rearrange("p (g c) -> p g c", c=3),
                        axis=AX.X, op=Op.add)
        v.tensor_scalar(da2, da2, 1e-12, None, Op.max)
        da = fb[:, N16:2 * N16]
        act.activation(da, da2, AF.Sqrt)
        dia = fc[:, :N16]
        v.reciprocal(dia, da)
        dsh = fc[:, N16:2 * N16]
        act.activation(dsh, da, AF.Sin, bias=PI, scale=-0.5)
        act.activation(gq[0][:, :N16], da, AF.Sin, bias=PI / 2, scale=-0.5)
        dk = fd[:, :N16]
        v.tensor_tensor(dk, dsh, dia, Op.mult)
        dv = dw_t[:].rearrange("p (g c) -> p g c", c=3)
        for i in range(3):
            v.tensor_tensor(gq[1 + i][:, :N16], dv[:, :, i], dk, Op.mult)

        # ---------------- streaming chunk loop ----------------
        # ACT is software-pipelined: Square-accum of chunk c is emitted
        # after Ln/Exp of chunk c+1 so it never blocks the next chunk.
        prev_u = None
        with tc.tile_pool(name="io", bufs=2) as iop, \
             tc.tile_pool(name="wk", bufs=2) as wkp:
            for c in range(NCH):
                csl = slice(c * CW, (c + 1) * CW)
                sd_t = iop.tile([P, CW], F16, name="sd_t", tag="sd")
                dma4(sd_t[:], sd_d[:, csl])
                gt_t = iop.tile([P, CW], F16, name="gt_t", tag="gt")
                dma4(gt_t[:], gt_d[:, csl])
                wh_t = iop.tile([P, CW], F16, name="wh_t", tag="wh")
                dma4(wh_t[:], wh_d[:, csl])
                mn_t = iop.tile([P, CW], F16, name="mn_t", tag="mn")
                dma4(mn_t[:], mn_d[:, csl])

                Sc = wkp.tile([P, CW], F16, name="Sc", tag="Sc")
                v.tensor_scalar(Sc[:], sd_t[:], 1e-3, None, Op.max)
                lnS = wkp.tile([P, CW], F32, name="lnS", tag="lnS")
                act.activation(lnS[:], Sc[:], AF.Ln,
                               accum_out=acc_ln[:, c:c + 1])
                isd = wkp.tile([P, CW], F16, name="isd", tag="isd")
                act.activation(isd[:], lnS[:], AF.Exp, scale=-1.0)
                d1 = wkp.tile([P, CW], F16, name="d1", tag="d1")
                g.tensor_tensor(d1[:], gt_t[:], wh_t[:], Op.subtract)
                dd = wkp.tile([P, CW], F16, name="dd", tag="dd")
                v.tensor_tensor(dd[:], d1[:], mn_t[:], Op.subtract)
                u = wkp.tile([P, CW], F16, name="u", tag="u")
                v.tensor_tensor(u[:], dd[:], isd[:], Op.mult)

                # w_hat 16-sums: direct TR-of-16 on DVE, even|odd split
                for i in range(3):
                    w16 = wh_t[:, i * CS:(i + 1) * CS].rearrange(
                        "p (gg k s) -> p gg k s", k=2, s=16)
                    ge = 16 * c
                    v.tensor_reduce(scat_v[:, ge:ge + 16, i],
                                    w16[:, :, 0, :], axis=AX.X, op=Op.add)
                    v.tensor_reduce(scat_v[:, 64 + ge:64 + ge + 16, i],
                                    w16[:, :, 1, :], axis=AX.X, op=Op.add)

                if prev_u is not None:
                    pc, pu, pj = prev_u
                    act.activation(pj[:], pu[:], AF.Square,
                                   accum_out=acc_u2a[:, pc:pc + 1])
                junka = wkp.tile([P, CW], F32, name="junka", tag="junka")
                prev_u = (c, u, junka)

            pc, pu, pj = prev_u
            act.activation(pj[:], pu[:], AF.Square,
                           accum_out=acc_u2a[:, pc:pc + 1])

        # ---------------- 32-level sums (even + odd halves) ---------------
        v.tensor_tensor(scat[:, 3 * N16:], scat[:, :3 * 64],
                        scat[:, 3 * 64:3 * N16], Op.add)

        # ---------------- hat quats: 5th-order Taylor ----------------
        n = NCAT
        sq = fa[:, :3 * n]
        act.activation(sq, scat[:], AF.Square)
        s2n = fb[:, :n]
        v.tensor_reduce(s2n, sq.rearrange("p (gg c) -> p gg c", c=3),
                        axis=AX.X, op=Op.add)
        h2 = fb[:, n:2 * n]
        v.tensor_scalar(h2, s2n, (DT / 2) ** 2, None, Op.mult)
        h4 = fc[:, :n]
        v.tensor_tensor(h4, h2, h2, Op.mult)
        t1 = fc[:, n:2 * n]
        v.tensor_scalar(t1, h2, -0.5, 1.0, Op.mult, Op.add)
        v.scalar_tensor_tensor(hq[0][:], h4, 1.0 / 24, t1, Op.mult, Op.add)
        v.tensor_scalar(t1, h2, -1.0 / 6, 1.0, Op.mult, Op.add)
        snc = fd[:, :n]
        v.scalar_tensor_tensor(snc, h4, 1.0 / 120, t1, Op.mult, Op.mult)
        for i in range(3):
            v.scalar_tensor_tensor(hq[1 + i][:], scat_v[:, :, i], DT / 2,
                                   snc, Op.mult, Op.mult)

        # ---------------- quaternion products ----------------
        Wc, Xc, Yc, Zc = 0, 1, 2, 3
        TERMS = {
            Wc: [(Wc, Wc), (Xc, Xc), (Yc, Yc), (Zc, Zc)],
            Xc: [(Wc, Xc), (Xc, Wc), (Yc, Zc), (Zc, Yc)],
            Yc: [(Wc, Yc), (Yc, Wc), (Zc, Xc), (Xc, Zc)],
            Zc: [(Wc, Zc), (Zc, Wc), (Xc, Yc), (Yc, Xc)],
        }

        def qmul(engs, outs, A, B, scr, conj_a=False):
            s = -1 if conj_a else 1
            signs = {
                Wc: [+1, -s, -s, -s],
                Xc: [+1, s, s, -s],
                Yc: [+1, s, s, -s],
                Zc: [+1, s, s, -s],
            }
            for oc, tl in TERMS.items():
                ve = engs[oc]
                ta, tb, tcs = scr[oc]
                sg = signs[oc]
                ve.tensor_tensor(ta, A[tl[0][0]], B[tl[0][1]], Op.mult)
                ve.tensor_tensor(tb, A[tl[1][0]], B[tl[1][1]], Op.mult)
                ve.tensor_tensor(ta, ta, tb,
                                 Op.add if sg[1] > 0 else Op.subtract)
                ve.tensor_tensor(tb, A[tl[2][0]], B[tl[2][1]], Op.mult)
                ve.tensor_tensor(tcs, A[tl[3][0]], B[tl[3][1]], Op.mult)
                ve.tensor_tensor(tb, tb, tcs,
                                 Op.add if sg[2] * sg[3] > 0 else Op.subtract)
                ve.tensor_tensor(outs[oc], ta, tb,
                                 Op.add if sg[2] > 0 else Op.subtract)

        dscr = (s16[0][:], s16[1][:], s16[2][:])
        pscr = (pxa[:], pxb[:], pxc[:])
        dscr64 = (s16[0][:, :64], s16[1][:, :64], s16[2][:, :64])
        # g32 = g16_even x g16_odd (contiguous halves)
        qmul({0: v, 1: v, 2: v, 3: v},
             [pl[:, N16:] for pl in gq],
             [pl[:, :64] for pl in gq],
             [pl[:, 64:N16] for pl in gq],
             {0: dscr64, 1: dscr64, 2: dscr64, 3: dscr64})
        # residual = conj(hat) x gt; x-comp on Pool with private scratch
        rout = [rqw[:], rqv[:, :n], rqv[:, n:2 * n], rqv[:, 2 * n:]]
        qmul({0: v, 1: g, 2: v, 3: v},
             rout, [pl[:] for pl in hq], [pl[:] for pl in gq],
             {0: dscr, 1: pscr, 2: dscr, 3: dscr}, conj_a=True)

        # ---------------- log: theta = 2*atan2(|v|, |w|) -------------------
        wf = fa[:, :n]
        act.activation(wf, rqw[:], AF.Abs)
        s0 = fa[:, n:2 * n]
        v.tensor_tensor(s0, wf, wf, Op.mult)
        s2t = fb[:, :n]
        v.tensor_scalar(s2t, s0, -1.0, 1.0, Op.mult, Op.add)
        v.tensor_scalar(s2t, s2t, 1e-12, None, Op.max)
        sv = fb[:, n:2 * n]
        act.activation(sv, s2t, AF.Sqrt)
        num = fc[:, :n]
        v.tensor_tensor(num, sv, wf, Op.min)
        den = fc[:, n:2 * n]
        v.tensor_tensor(den, sv, wf, Op.max)
        idn = fd[:, :n]
        v.reciprocal(idn, den)
        v.tensor_tensor(num, num, idn, Op.mult)
        at = fd[:, n:2 * n]
        act.activation(at, num, AF.Arctan)
        sel = s0
        v.tensor_tensor(sel, sv, wf, Op.is_gt)
        uu = fc[:, :n]
        v.tensor_scalar(uu, at, -2.0, PI / 2, Op.mult, Op.add)
        v.tensor_tensor(uu, uu, sel, Op.mult)
        th2 = fc[:, n:2 * n]
        v.tensor_tensor(th2, at, uu, Op.add)
        iss = fa[:, :n]
        v.reciprocal(iss, sv)
        gf = fa[:, n:2 * n]
        v.scalar_tensor_tensor(gf, th2, 2.0 / H_, iss, Op.mult, Op.mult)
        v.tensor_tensor(gf, gf, mkc_t[:], Op.mult)

        # ---------------- huber on concatenated comps ----------------
        tvc = fb[:, :3 * n]
        for i in range(3):
            v.tensor_tensor(tvc[:, i * n:(i + 1) * n], gf,
                            rqv[:, i * n:(i + 1) * n], Op.mult)
        abc = fa[:, :3 * n]
        act.activation(abc, tvc, AF.Abs)
        mq3 = pp.tile([P, 3 * NCAT], F32, name="mq3", tag="mq3")
        hm3 = pp.tile([P, 3 * NCAT], F32, name="hm3", tag="hm3")
        v.tensor_scalar(mq3[:], abc, 1.0, None, Op.min)
        v.scalar_tensor_tensor(hm3[:], abc, 2.0, mq3[:], Op.mult, Op.subtract)
        v.tensor_tensor(hm3[:], hm3[:], mq3[:], Op.mult)
        hm_v = hm3[:].rearrange("p (c gg) -> p c gg", gg=n)
        v.tensor_reduce(acc16[:], hm_v[:, :, :N16], axis=AX.X, op=Op.add)
        v.tensor_reduce(acc32[:], hm_v[:, :, N16:], axis=AX.X, op=Op.add)

        out_t = pp.tile([P, 4], F32, name="out_t", tag="out_t")
        v.tensor_reduce(out_t[:, 0:1], acc16[:], axis=AX.X, op=Op.add)
        v.tensor_reduce(out_t[:, 1:2], acc32[:], axis=AX.X, op=Op.add)
        v.tensor_reduce(out_t[:, 2:3], acc_ln[:], axis=AX.X, op=Op.add)
        v.tensor_reduce(out_t[:, 3:4], acc_u2a[:], axis=AX.X, op=Op.add)
        nc.sync.dma_start(out=out_d[:], in_=out_t[:])

    return nc


def combine(parts):
    """parts: [n_cores, P, 4] per-partition sums."""
    s = np.asarray(parts, dtype=np.float64).reshape(-1, 4).sum(axis=0)
    n16, n32 = T_FULL // 16, T_FULL // 32
    gyro16 = W_ * H_ ** 2 * 0.5 * s[0] / (N_FULL * (n16 - N0) * 3)
    gyro32 = (W_ * H_ ** 2 / 4) * 0.5 * s[1] / (N_FULL * (n32 - N0) * 3)
    gnll = (2.0 * s[2] + s[3]) / (2.0 * N_FULL * T_FULL * 3)
    return np.array(gyro16 + gyro32 + gnll, dtype=np.float32)


_NC_CACHE = {}


def last_exec_time_ns():
    res = _NC_CACHE.get("last_res")
    if res is None:
        return None
    return res.exec_time_ns or res.mean_exec_time_ns


# group permutation: even groups first, then odd (within each partition)
_GPERM = np.concatenate([np.arange(0, N16, 2), np.arange(1, N16, 2)])


def make_maskc():
    """[P, NCAT] f32; zero the first N0 16-groups and 32-groups of each
    sequence (they live on partitions == 0 mod SP, in t-order)."""
    mk16 = np.ones((P, N16), dtype=np.float32)
    mk16[::SP, :N0] = 0.0
    mk16 = mk16[:, _GPERM]          # even|odd column order
    mk32 = np.ones((P, N32), dtype=np.float32)
    mk32[::SP, :N0] = 0.0
    return np.ascontiguousarray(np.concatenate([mk16, mk32], axis=1))


def _prep_stream(shard):
    """[NSEQ, T, 3] f32 -> [P, NCH*3*CS] fp16, chunk-plane layout."""
    a = shard.reshape(NSEQ, SP, NCH, CS, 3).transpose(0, 1, 2, 4, 3)
    return np.ascontiguousarray(a.reshape(P, NCH * 3 * CS).astype(np.float16))


def _prep_dw(shard):
    """[NSEQ, T, 3] f32 -> [P, 3*N16] fp16 interleaved, even|odd groups."""
    a = shard[:, ::16]                      # [NSEQ, L16=1024, 3]
    a = a.reshape(NSEQ, SP, N16, 3)[:, :, _GPERM]
    return np.ascontiguousarray(a.reshape(P, 3 * N16).astype(np.float16))


def _register_ntff_shim():
    import sys, types
    try:
        import antenv.axon_hooks  # noqa: F401
        return
    except ImportError:
        pass
    from trn_agent_boot.trn_boot import _ntff_profile_via_ctypes
    hook = _ntff_profile_via_ctypes('/opt/axon/libaxon_pjrt.so')
    mod = types.ModuleType("antenv.axon_hooks")
    mod.get_axon_ntff_profile_hook = lambda: hook
    import antenv
    antenv.axon_hooks = mod
    sys.modules["antenv.axon_hooks"] = mod


def kernel(w_hat, dw_16, w_gt, w_mean, w_std):
    import os
    from concourse.bass_utils import run_bass_kernel_spmd
    if os.environ.get("KERNEL_PROFILE"):
        _register_ntff_shim()

    if "nc" not in _NC_CACHE:
        nc_ = build()
        _split_multi_waits(nc_)
        _NC_CACHE["nc"] = nc_
    nc = _NC_CACHE["nc"]

    mkc = make_maskc()
    spc = N_FULL // N_CORES
    arrs = {"w_hat": np.asarray(w_hat, np.float32),
            "dw_16": np.asarray(dw_16, np.float32),
            "w_gt": np.asarray(w_gt, np.float32),
            "w_mean": np.asarray(w_mean, np.float32),
            "w_std": np.asarray(w_std, np.float32)}
    in_maps = []
    for c in range(N_CORES):
        sl = slice(c * spc, (c + 1) * spc)
        m = {k: _prep_stream(a[sl]) for k, a in arrs.items() if k != "dw_16"}
        m["dw_16"] = _prep_dw(arrs["dw_16"][sl])
        m["maskc"] = mkc
        in_maps.append(m)
    res = run_bass_kernel_spmd(nc, in_maps, list(range(N_CORES)),
                               trace=bool(os.environ.get("KERNEL_PROFILE")))
    _NC_CACHE["last_res"] = res
    parts = np.stack([r["out"] for r in res.results])
    return combine(parts)


# revision 20
# speedup vs baseline: 1.1421x; 1.1421x over previous
"""Trainium2 Bass kernel for nn_DGLossVer2 (gyro Huber loss + gaussian NLL).

Strategy (v2)
-------------
Data-parallel over batch N=128 across 8 NeuronCores (16 sequences/core,
8 partitions per sequence, L=2048 steps per partition).

Host prep (layout/dtype only, no arithmetic): every stream is cast to
fp16 and laid out planar per chunk: [P, nch, 3, Cs].  dw_16 is
subsampled host-side (dw_16[:, ::16], pure indexing) and its 16-groups
are stored even|odd-split so the level-5 pairwise quat product reads
contiguous halves (keeps DVE 2x_1p mode).  fp16 end-to-end was
validated against the f32 reference in numpy at rel err 2.8e-5.

Engine split per chunk (Cs=512 steps, 1536 elems):
  DVE : Sc=max(sd,1e-3); d1=gt-wh; d=d1-mn; u=d*isd (all fp16 2x);
        level-3/4 tree reduce (TR of 4); TTR partial sum of u^2
  ACT : lnS=Ln(Sc) [+accum]; isd=Exp(-lnS); Square-accum of u^2 part
  Pool: pairwise halve levels 1+2 of the w_hat 16-sum tree
Tail: hat quats by 5th-order Taylor (f32 math, fp16 storage); gt quats
from dw (Sqrt/Sin); q32 = q16e*q16o and residual conj(hat)*gt in fp16
(2x); theta = 2*atan2(|v|, w) with w=cos(theta/2)>0 always, via min/max
reflection; Huber m*(2|t|-m) partial-summed with tensor_tensor_reduce.
Host combines per-partition partials in f64.
"""

import numpy as np

import concourse.bass as bass
import concourse.mybir as mybir
from concourse.mybir import AluOpType as Op
from concourse.mybir import ActivationFunctionType as AF
from concourse.tile import TileContext

F32 = mybir.dt.float32
F16 = mybir.dt.float16
AX = mybir.AxisListType


def _patch_drain():
    """walrus codegen in this container rejects >1 sync wait on SP-engine
    instructions; spread the kernel-tail drain's waits across 1-wait NOPs."""
    from concourse import tile as tile_mod
    from concourse.vector_clock import ScopedClock

    if getattr(tile_mod.TileContext, "_drain_patched", False):
        return

    def _drain_and_barrier(self, tick_clock, wait_clock):
        nop0 = self.nc.sync.nop(nofuse=True)
        wait_clock.add_sem_waits(nop0.ins,
                                 ScopedClock({None: tick_clock.global_clock}))
        si = nop0.ins.sync_info
        if si is not None and len(si.on_wait) > 1:
            waits = list(si.on_wait)
            si.on_wait = waits[:1]
            for w in waits[1:]:
                nopn = self.nc.sync.nop(nofuse=True)
                nopn.ins.sync_info = mybir.SyncInfo(on_wait=[w], on_update=[])
        self.nc.sync.drain()
        self.nc.all_engine_barrier()
        assert self.sems is not None
        popped = self.nc._tile_sem_poison_stack.pop()
        assert popped is self._sem_poison
        self.nc.clear_and_free_semaphores(list(self.sems.allocated().values()))
        self.nc.all_engine_barrier()

    tile_mod.TileContext._drain_and_barrier = _drain_and_barrier
    tile_mod.TileContext._drain_patched = True


def _split_multi_waits(nc):
    """This container's walrus codegen allows only one sync wait per
    instruction; move extra waits onto same-engine NoOps inserted before."""
    n = 0
    for bb in nc.m.functions[0].blocks:
        new = []
        for inst in bb.instructions:
            si = inst.sync_info
            if si is not None and len(si.on_wait) > 1:
                waits = list(si.on_wait)
                for w in waits[:-1]:
                    n += 1
                    new.append(mybir.InstNoOp(
                        name=f"wsplit-{n}", engine=inst.engine,
                        sync_info=mybir.SyncInfo(on_wait=[w], on_update=[]),
                        bass_nofuse=True))
                si.on_wait = waits[-1:]
            new.append(inst)
        bb.instructions[:] = new
    return n


DT = 0.005
W_ = 1.0e6
H_ = 0.005
N0 = 5
PI = float(np.pi)

N_CORES = 8
N_FULL, T_FULL = 128, 16384
P = 128
NSEQ = 16               # sequences per core
SP = P // NSEQ          # partitions per sequence (8)
L = T_FULL // SP        # steps per partition (2048)
NCH = 4                 # chunks
CS = L // NCH           # steps per partition per chunk (512)
N16 = L // 16           # 128 16-groups per partition
N32 = L // 32           # 64
NCAT = N16 + N32        # 192


def build():
    _patch_drain()
    nc = bass.Bass()
    for cname, cval in (("pi2", PI / 2), ("pi", PI), ("tiny", 1e-30),
                        ("m1", -1.0)):
        _cc = nc.alloc_sbuf_tensor(f"const-f32-{cname}", [128, 1], F32)
        nc.gpsimd.memset(_cc.ap(), cval)
        nc.const_aps.aps[(F32, cval)] = _cc.ap()
    nc.all_engine_barrier()

    CW = 3 * CS          # elems per chunk (1536)
    wh_d = nc.declare_dram_parameter("w_hat", [P, NCH * CW], F16, isOutput=False)
    gt_d = nc.declare_dram_parameter("w_gt", [P, NCH * CW], F16, isOutput=False)
    mn_d = nc.declare_dram_parameter("w_mean", [P, NCH * CW], F16, isOutput=False)
    sd_d = nc.declare_dram_parameter("w_std", [P, NCH * CW], F16, isOutput=False)
    dw_d = nc.declare_dram_parameter("dw_16", [P, 3 * N16], F16, isOutput=False)
    mkc_d = nc.declare_dram_parameter("maskc", [P, NCAT], F32, isOutput=False)
    out_d = nc.declare_dram_parameter("out", [P, 4], F32, isOutput=True)

    from contextlib import ExitStack
    with TileContext(nc) as tc, ExitStack() as _es:
        v = nc.vector
        g = nc.gpsimd
        act = nc.scalar
        pp = _es.enter_context(tc.tile_pool(name="persist", bufs=1))

        def ptile(shape, name, dtype=F32):
            return pp.tile(shape, dtype, name=name, tag=name)

        # persistent planes
        # scat: interleaved [g, c] 16|32-group sums (f32)
        scat = ptile([P, 3 * NCAT], "scat")
        scat_v = scat[:].rearrange("p (g c) -> p g c", c=3)
        dw_t = ptile([P, 3 * N16], "dw_t", F16)
        gq = [ptile([P, NCAT], f"gq{i}", F16) for i in range(4)]  # gt quats
        hq = [ptile([P, NCAT], f"hq{i}", F16) for i in range(4)]  # hat quats
        rqw = ptile([P, NCAT], "rqw", F16)          # residual w
        rqv = ptile([P, 3 * NCAT], "rqv", F16)      # residual xyz comp-major
        s16 = [ptile([P, NCAT], f"s16_{i}", F16) for i in range(3)]  # qmul scr
        mkc_t = ptile([P, NCAT], "mkc")
        acc_ln = ptile([P, NCH], "acc_ln")
        acc_u2a = ptile([P, NCH], "acc_u2a")
        acc16 = ptile([P, 3], "acc16")
        acc32 = ptile([P, 3], "acc32")
        # f32 scratch planes; pxa..pxc are Pool-private
        fa = ptile([P, 3 * NCAT], "fa")
        fb = ptile([P, 3 * NCAT], "fb")
        fc = ptile([P, 2 * NCAT], "fc")
        fd = ptile([P, 2 * NCAT], "fd")
        pxa = ptile([P, NCAT], "pxa")
        pxb = ptile([P, NCAT], "pxb")
        pxc = ptile([P, NCAT], "pxc")

        nc.sync.dma_start(out=mkc_t[:], in_=mkc_d[:])
        nc.sync.dma_start(out=dw_t[:], in_=dw_d[:])

        def dma4(tile_ap, dram_ap, k=1):
            step = P // k
            for i_ in range(k):
                psl = slice(i_ * step, (i_ + 1) * step)
                nc.sync.dma_start(out=tile_ap[psl, :], in_=dram_ap[psl, :])

        # ------------- dw -> gt quats (before chunk loop; sqrt+trig) -------
        dsq = fa[:, :3 * N16]
        v.tensor_tensor(dsq, dw_t[:], dw_t[:], Op.mult)
        da2 = fb[:, :N16]
        v.tensor_reduce(da2, dsq.rearrange("p (g c) -> p g c", c=3),
                        axis=AX.X, op=Op.add)
        v.tensor_scalar(da2, da2, 1e-12, None, Op.max)
        da = fb[:, N16:2 * N16]
        act.activation(da, da2, AF.Sqrt)
        dia = fc[:, :N16]
        v.reciprocal(dia, da)
        dsh = fc[:, N16:2 * N16]
        act.activation(dsh, da, AF.Sin, bias=PI, scale=-0.5)
        act.activation(gq[0][:, :N16], da, AF.Sin, bias=PI / 2, scale=-0.5)
        dk = fd[:, :N16]
        v.tensor_tensor(dk, dsh, dia, Op.mult)
        dv = dw_t[:].rearrange("p (g c) -> p g c", c=3)
        for i in range(3):
            v.tensor_tensor(gq[1 + i][:, :N16], dv[:, :, i], dk, Op.mult)

        # ---------------- streaming chunk loop ----------------
        # ACT is software-pipelined: Square-accum of chunk c is emitted
        # after Ln/Exp of chunk c+1 so it never blocks the next chunk.
        prev_u = None
        with tc.tile_pool(name="io", bufs=2) as iop, \
             tc.tile_pool(name="wk", bufs=2) as wkp:
            for c in range(NCH):
                csl = slice(c * CW, (c + 1) * CW)
                sd_t = iop.tile([P, CW], F16, name="sd_t", tag="sd")
                dma4(sd_t[:], sd_d[:, csl])
                gt_t = iop.tile([P, CW], F16, name="gt_t", tag="gt")
                dma4(gt_t[:], gt_d[:, csl])
                wh_t = iop.tile([P, CW], F16, name="wh_t", tag="wh")
                dma4(wh_t[:], wh_d[:, csl])
                mn_t = iop.tile([P, CW], F16, name="mn_t", tag="mn")
                dma4(mn_t[:], mn_d[:, csl])

                Sc = wkp.tile([P, CW], F16, name="Sc", tag="Sc")
                v.tensor_scalar(Sc[:], sd_t[:], 1e-3, None, Op.max)
                lnS = wkp.tile([P, CW], F32, name="lnS", tag="lnS")
                act.activation(lnS[:], Sc[:], AF.Ln,
                               accum_out=acc_ln[:, c:c + 1])
                isd = wkp.tile([P, CW], F16, name="isd", tag="isd")
                act.activation(isd[:], lnS[:], AF.Exp, scale=-1.0)
                d1 = wkp.tile([P, CW], F16, name="d1", tag="d1")
                g.tensor_tensor(d1[:], gt_t[:], wh_t[:], Op.subtract)
                dd = wkp.tile([P, CW], F16, name="dd", tag="dd")
                v.tensor_tensor(dd[:], d1[:], mn_t[:], Op.subtract)
                u = wkp.tile([P, CW], F16, name="u", tag="u")
                v.tensor_tensor(u[:], dd[:], isd[:], Op.mult)

                # w_hat 16-sums: direct TR-of-16 on DVE, even|odd split
                for i in range(3):
                    w16 = wh_t[:, i * CS:(i + 1) * CS].rearrange(
                        "p (gg k s) -> p gg k s", k=2, s=16)
                    ge = 16 * c
                    v.tensor_reduce(scat_v[:, ge:ge + 16, i],
                                    w16[:, :, 0, :], axis=AX.X, op=Op.add)
                    v.tensor_reduce(scat_v[:, 64 + ge:64 + ge + 16, i],
                                    w16[:, :, 1, :], axis=AX.X, op=Op.add)

                if prev_u is not None:
                    pc, pu, pj = prev_u
                    act.activation(pj[:], pu[:], AF.Square,
                                   accum_out=acc_u2a[:, pc:pc + 1])
                junka = wkp.tile([P, CW], F32, name="junka", tag="junka")
                prev_u = (c, u, junka)

            pc, pu, pj = prev_u
            act.activation(pj[:], pu[:], AF.Square,
                           accum_out=acc_u2a[:, pc:pc + 1])

        # ---------------- 32-level sums (even + odd halves) ---------------
        v.tensor_tensor(scat[:, 3 * N16:], scat[:, :3 * 64],
                        scat[:, 3 * 64:3 * N16], Op.add)

        # ---------------- hat quats: 5th-order Taylor ----------------
        n = NCAT
        sq = fa[:, :3 * n]
        act.activation(sq, scat[:], AF.Square)
        s2n = fb[:, :n]
        v.tensor_reduce(s2n, sq.rearrange("p (gg c) -> p gg c", c=3),
                        axis=AX.X, op=Op.add)
        h2 = fb[:, n:2 * n]
        v.tensor_scalar(h2, s2n, (DT / 2) ** 2, None, Op.mult)
        h4 = fc[:, :n]
        v.tensor_tensor(h4, h2, h2, Op.mult)
        t1 = fc[:, n:2 * n]
        v.tensor_scalar(t1, h2, -0.5, 1.0, Op.mult, Op.add)
        v.scalar_tensor_tensor(hq[0][:], h4, 1.0 / 24, t1, Op.mult, Op.add)
        v.tensor_scalar(t1, h2, -1.0 / 6, 1.0, Op.mult, Op.add)
        snc = fd[:, :n]
        v.scalar_tensor_tensor(snc, h4, 1.0 / 120, t1, Op.mult, Op.mult)
        for i in range(3):
            v.scalar_tensor_tensor(hq[1 + i][:], scat_v[:, :, i], DT / 2,
                                   snc, Op.mult, Op.mult)

        # ---------------- quaternion products ----------------
        Wc, Xc, Yc, Zc = 0, 1, 2, 3
        TERMS = {
            Wc: [(Wc, Wc), (Xc, Xc), (Yc, Yc), (Zc, Zc)],
            Xc: [(Wc, Xc), (Xc, Wc), (Yc, Zc), (Zc, Yc)],
            Yc: [(Wc, Yc), (Yc, Wc), (Zc, Xc), (Xc, Zc)],
            Zc: [(Wc, Zc), (Zc, Wc), (Xc, Yc), (Yc, Xc)],
        }

        def qmul(engs, outs, A, B, scr, conj_a=False):
            s = -1 if conj_a else 1
            signs = {
                Wc: [+1, -s, -s, -s],
                Xc: [+1, s, s, -s],
                Yc: [+1, s, s, -s],
                Zc: [+1, s, s, -s],
            }
            for oc, tl in TERMS.items():
                ve = engs[oc]
                ta, tb, tcs = scr[oc]
                sg = signs[oc]
                ve.tensor_tensor(ta, A[tl[0][0]], B[tl[0][1]], Op.mult)
                ve.tensor_tensor(tb, A[tl[1][0]], B[tl[1][1]], Op.mult)
                ve.tensor_tensor(ta, ta, tb,
                                 Op.add if sg[1] > 0 else Op.subtract)
                ve.tensor_tensor(tb, A[tl[2][0]], B[tl[2][1]], Op.mult)
                ve.tensor_tensor(tcs, A[tl[3][0]], B[tl[3][1]], Op.mult)
                ve.tensor_tensor(tb, tb, tcs,
                                 Op.add if sg[2] * sg[3] > 0 else Op.subtract)
                ve.tensor_tensor(outs[oc], ta, tb,
                                 Op.add if sg[2] > 0 else Op.subtract)

        dscr = (s16[0][:], s16[1][:], s16[2][:])
        pscr = (pxa[:], pxb[:], pxc[:])
        dscr64 = (s16[0][:, :64], s16[1][:, :64], s16[2][:, :64])
        # g32 = g16_even x g16_odd (contiguous halves)
        qmul({0: v, 1: v, 2: v, 3: v},
             [pl[:, N16:] for pl in gq],
             [pl[:, :64] for pl in gq],
             [pl[:, 64:N16] for pl in gq],
             {0: dscr64, 1: dscr64, 2: dscr64, 3: dscr64})
        # residual = conj(hat) x gt; x-comp on Pool with private scratch
        rout = [rqw[:], rqv[:, :n], rqv[:, n:2 * n], rqv[:, 2 * n:]]
        qmul({0: v, 1: g, 2: v, 3: v},
             rout, [pl[:] for pl in hq], [pl[:] for pl in gq],
             {0: dscr, 1: pscr, 2: dscr, 3: dscr}, conj_a=True)

        # ---------------- log: theta = 2*atan2(|v|, |w|) -------------------
        wf = fa[:, :n]
        act.activation(wf, rqw[:], AF.Abs)
        s0 = fa[:, n:2 * n]
        v.tensor_tensor(s0, wf, wf, Op.mult)
        s2t = fb[:, :n]
        v.tensor_scalar(s2t, s0, -1.0, 1.0, Op.mult, Op.add)
        v.tensor_scalar(s2t, s2t, 1e-12, None, Op.max)
        sv = fb[:, n:2 * n]
        act.activation(sv, s2t, AF.Sqrt)
        num = fc[:, :n]
        v.tensor_tensor(num, sv, wf, Op.min)
        den = fc[:, n:2 * n]
        v.tensor_tensor(den, sv, wf, Op.max)
        idn = fd[:, :n]
        v.reciprocal(idn, den)
        v.tensor_tensor(num, num, idn, Op.mult)
        at = fd[:, n:2 * n]
        act.activation(at, num, AF.Arctan)
        sel = s0
        v.tensor_tensor(sel, sv, wf, Op.is_gt)
        uu = fc[:, :n]
        v.tensor_scalar(uu, at, -2.0, PI / 2, Op.mult, Op.add)
        v.tensor_tensor(uu, uu, sel, Op.mult)
        th2 = fc[:, n:2 * n]
        v.tensor_tensor(th2, at, uu, Op.add)
        iss = fa[:, :n]
        v.reciprocal(iss, sv)
        gf = fa[:, n:2 * n]
        v.scalar_tensor_tensor(gf, th2, 2.0 / H_, iss, Op.mult, Op.mult)
        v.tensor_tensor(gf, gf, mkc_t[:], Op.mult)

        # ---------------- huber on concatenated comps ----------------
        tvc = fb[:, :3 * n]
        for i in range(3):
            v.tensor_tensor(tvc[:, i * n:(i + 1) * n], gf,
                            rqv[:, i * n:(i + 1) * n], Op.mult)
        abc = fa[:, :3 * n]
        act.activation(abc, tvc, AF.Abs)
        mq3 = pp.tile([P, 3 * NCAT], F32, name="mq3", tag="mq3")
        hm3 = pp.tile([P, 3 * NCAT], F32, name="hm3", tag="hm3")
        v.tensor_scalar(mq3[:], abc, 1.0, None, Op.min)
        v.scalar_tensor_tensor(hm3[:], abc, 2.0, mq3[:], Op.mult, Op.subtract)
        v.tensor_tensor(hm3[:], hm3[:], mq3[:], Op.mult)
        hm_v = hm3[:].rearrange("p (c gg) -> p c gg", gg=n)
        v.tensor_reduce(acc16[:], hm_v[:, :, :N16], axis=AX.X, op=Op.add)
        v.tensor_reduce(acc32[:], hm_v[:, :, N16:], axis=AX.X, op=Op.add)

        out_t = pp.tile([P, 4], F32, name="out_t", tag="out_t")
        v.tensor_reduce(out_t[:, 0:1], acc16[:], axis=AX.X, op=Op.add)
        v.tensor_reduce(out_t[:, 1:2], acc32[:], axis=AX.X, op=Op.add)
        v.tensor_reduce(out_t[:, 2:3], acc_ln[:], axis=AX.X, op=Op.add)
        v.tensor_reduce(out_t[:, 3:4], acc_u2a[:], axis=AX.X, op=Op.add)
        nc.sync.dma_start(out=out_d[:], in_=out_t[:])

    return nc


def combine(parts):
    """parts: [n_cores, P, 4] per-partition sums."""
    s = np.asarray(parts, dtype=np.float64).reshape(-1, 4).sum(axis=0)
    n16, n32 = T_FULL // 16, T_FULL // 32
    gyro16 = W_ * H_ ** 2 * 0.5 * s[0] / (N_FULL * (n16 - N0) * 3)
    gyro32 = (W_ * H_ ** 2 / 4) * 0.5 * s[1] / (N_FULL * (n32 - N0) * 3)
    gnll = (2.0 * s[2] + s[3]) / (2.0 * N_FULL * T_FULL * 3)
    return np.array(gyro16 + gyro32 + gnll, dtype=np.float32)


_NC_CACHE = {}


def last_exec_time_ns():
    res = _NC_CACHE.get("last_res")
    if res is None:
        return None
    return res.exec_time_ns or res.mean_exec_time_ns


# group permutation: even groups first, then odd (within each partition)
_GPERM = np.concatenate([np.arange(0, N16, 2), np.arange(1, N16, 2)])


def make_maskc():
    """[P, NCAT] f32; zero the first N0 16-groups and 32-groups of each
    sequence (they live on partitions == 0 mod SP, in t-order)."""
    mk16 = np.ones((P, N16), dtype=np.float32)
    mk16[::SP, :N0] = 0.0
    mk16 = mk16[:, _GPERM]          # even|odd column order
    mk32 = np.ones((P, N32), dtype=np.float32)
    mk32[::SP, :N0] = 0.0
    return np.ascontiguousarray(np.concatenate([mk16, mk32], axis=1))


def _prep_stream(shard):
    """[NSEQ, T, 3] f32 -> [P, NCH*3*CS] fp16, chunk-plane layout."""
    a = shard.reshape(NSEQ, SP, NCH, CS, 3).transpose(0, 1, 2, 4, 3)
    return np.ascontiguousarray(a.reshape(P, NCH * 3 * CS).astype(np.float16))


def _prep_dw(shard):
    """[NSEQ, T, 3] f32 -> [P, 3*N16] fp16 interleaved, even|odd groups."""
    a = shard[:, ::16]                      # [NSEQ, L16=1024, 3]
    a = a.reshape(NSEQ, SP, N16, 3)[:, :, _GPERM]
    return np.ascontiguousarray(a.reshape(P, 3 * N16).astype(np.float16))


def _register_ntff_shim():
    import sys, types
    try:
        import antenv.axon_hooks  # noqa: F401
        return
    except ImportError:
        pass
    from trn_agent_boot.trn_boot import _ntff_profile_via_ctypes
    hook = _ntff_profile_via_ctypes('/opt/axon/libaxon_pjrt.so')
    mod = types.ModuleType("antenv.axon_hooks")
    mod.get_axon_ntff_profile_hook = lambda: hook
    import antenv
    antenv.axon_hooks = mod
    sys.modules["antenv.axon_hooks"] = mod


def kernel(w_hat, dw_16, w_gt, w_mean, w_std):
    import os
    from concourse.bass_utils import run_bass_kernel_spmd
    if os.environ.get("KERNEL_PROFILE"):
        _register_ntff_shim()

    if "nc" not in _NC_CACHE:
        nc_ = build()
        _split_multi_waits(nc_)
        _NC_CACHE["nc"] = nc_
    nc = _NC_CACHE["nc"]

    mkc = make_maskc()
    spc = N_FULL // N_CORES
    arrs = {"w_hat": np.asarray(w_hat, np.float32),
            "dw_16": np.asarray(dw_16, np.float32),
            "w_gt": np.asarray(w_gt, np.float32),
            "w_mean": np.asarray(w_mean, np.float32),
            "w_std": np.asarray(w_std, np.float32)}
    in_maps = []
    for c in range(N_CORES):
        sl = slice(c * spc, (c + 1) * spc)
        m = {k: _prep_stream(a[sl]) for k, a in arrs.items() if k != "dw_16"}
        m["dw_16"] = _prep_dw(arrs["dw_16"][sl])
        m["maskc"] = mkc
        in_maps.append(m)
    res = run_bass_kernel_spmd(nc, in_maps, list(range(N_CORES)),
                               trace=bool(os.environ.get("KERNEL_PROFILE")))
    _NC_CACHE["last_res"] = res
    parts = np.stack([r["out"] for r in res.results])
    return combine(parts)
